# revision 9
# baseline (speedup 1.0000x reference)
"""Trainium2 kernel for nn_Conv_RBS_density.

The reference applies 48 sequential RBS-gate conjugations
``rho <- U rho U^T`` where every ``U = cos(t)*A + sin(t)*B + C`` is an
orthogonal matrix of 30 disjoint 2x2 Givens rotations.  By associativity
the whole pipeline is ``out = V rho V^T`` with ``V = U48 @ ... @ U1``.
V is accumulated on the host with sparse Givens row updates
(O(48*30*N) flops — negligible); the O(N^3) work — the dense matmuls
against rho — runs on the 8 NeuronCores.

Structure: gates only couple basis states within connected components of
the qubit-tile graph — V is block-diagonal (28 components of 16 states +
8 of 6).  Packing components into 4 bins of exactly 124 states gives a
grouped order where V_g = diag(B0, B1, B2, B3), each 124x124.

Sharding: grouped output rows split 8 x 62; core k (block B = k//2) does
  mm1:  P_k[j]  = rho_g[Brows, jcols].T @ V_g[gk, Bcols].T   (4x [124,62])
        (= (V_k rho_g)^T chunks, using rho symmetry)
  mm2:  out_k[:, jcols] = P_k[j].T @ B_j^T                   (4x [62,124])
No collectives; the host concatenates rows and un-permutes.
"""

import numpy as np

import concourse.mybir as mybir
from concourse import bacc
from concourse.bass import ts
from concourse.bass_utils import run_bass_kernel_spmd
from concourse.tile import TileContext

N = 496          # C(32, 2) Hamming-weight-2 states
NCORES = 8
R = N // NCORES  # 62 output rows per core
BK = 124         # block size
NB = N // BK     # 4 blocks

_cache = {}


USE_F32R = True


def _build_program():
    nc = bacc.Bacc(
        "TRN2", target_bir_lowering=False, debug=False, num_devices=NCORES
    )
    f32 = mybir.dt.float32
    mmdt = mybir.dt.float32r if USE_F32R else f32
    rho_d = nc.dram_tensor("rho", [BK, N], mmdt, kind="ExternalInput")
    vkt_d = nc.dram_tensor("vkt", [BK, R], mmdt, kind="ExternalInput")
    vtb_d = nc.dram_tensor("vtb", [BK, N], mmdt, kind="ExternalInput")
    out_d = nc.dram_tensor("out", [R, N], f32, kind="ExternalOutput")

    qs = [nc.sync, nc.scalar, nc.gpsimd]

    with TileContext(nc) as tc:
        with (
            tc.tile_pool(name="sbuf", bufs=1) as sbuf,
            tc.tile_pool(name="psum", bufs=1, space="PSUM") as psum,
        ):
            # mm1 inputs first (vkt + rho chunks), vtb after
            vkt_sb = sbuf.tile([BK, R], mmdt, tag="vkt")
            qs[0].dma_start(vkt_sb[:], vkt_d[:, :])
            rho_sb = []
            for j in range(NB):
                r = sbuf.tile([BK, BK], mmdt, tag=f"rho{j}")
                qs[(j + 1) % 3].dma_start(r[:], rho_d[:, ts(j, BK)])
                rho_sb.append(r)
            vtb_sb = []
            for j in range(NB):
                w = sbuf.tile([BK, BK], mmdt, tag=f"vtb{j}")
                qs[(j + 2) % 3].dma_start(w[:], vtb_d[:, ts(j, BK)])
                vtb_sb.append(w)

            # mm1: P chunks, then copy PSUM->SBUF for use as mm2 lhsT
            pk_sb = []
            for j in range(NB):
                pp = psum.tile([BK, R], f32, tag=f"pp{j}")
                nc.tensor.matmul(pp[:], rho_sb[j][:], vkt_sb[:], start=True, stop=True)
                pk = sbuf.tile([BK, R], mmdt, tag=f"pk{j}")
                nc.vector.tensor_copy(pk[:], pp[:])
                pk_sb.append(pk)

            # mm2: out column chunks; copy to SBUF then DMA out per chunk
            for j in range(NB):
                po = psum.tile([R, BK], f32, tag=f"po{j}")
                nc.tensor.matmul(po[:], pk_sb[j][:], vtb_sb[j][:], start=True, stop=True)
                ob = sbuf.tile([R, BK], f32, tag=f"ob{j}")
                nc.vector.tensor_copy(ob[:], po[:])
                qs[j % 3].dma_start(out_d[:, ts(j, BK)], ob[:])

    nc.compile()
    return nc


def _program():
    if "nc" not in _cache:
        _cache["nc"] = _build_program()
    return _cache["nc"]


def _gate_pairs(B_stack):
    """Per unique gate: (s, q) index arrays with B[u, s, q] = +1."""
    pairs = []
    for u in range(B_stack.shape[0]):
        pos = np.argwhere(B_stack[u] > 0.5)
        pairs.append((pos[:, 0], pos[:, 1]))
    return pairs


def _build_V(thetas, pairs, u_idx, p_idx, n):
    """V = U_G ... U_1 via sparse Givens row updates (float64)."""
    thetas = np.asarray(thetas, np.float64)
    cos_t, sin_t = np.cos(thetas), np.sin(thetas)
    V = np.eye(n)
    for g in range(len(u_idx)):
        u, p = int(u_idx[g]), int(p_idx[g])
        c, s = cos_t[p], sin_t[p]
        S, Q = pairs[u]
        vs, vq = V[S], V[Q]
        V[S] = c * vs + s * vq
        V[Q] = -s * vs + c * vq
    return V


def _grouping(pairs, n):
    """Union states coupled by any gate; pack components into NB bins of BK."""
    parent = list(range(n))

    def find(a):
        while parent[a] != a:
            parent[a] = parent[parent[a]]
            a = parent[a]
        return a

    for S, Q in pairs:
        for s, q in zip(S.tolist(), Q.tolist()):
            ra, rb = find(s), find(q)
            if ra != rb:
                parent[ra] = rb

    comps = {}
    for i in range(n):
        comps.setdefault(find(i), []).append(i)
    comps = sorted(comps.values(), key=len, reverse=True)

    bins = [[] for _ in range(NB)]
    for comp in comps:
        for b in bins:
            if len(b) + len(comp) <= BK:
                b.extend(comp)
                break
        else:
            raise ValueError("component packing failed")
    assert all(len(b) == BK for b in bins), [len(b) for b in bins]
    return np.array([i for b in bins for i in b], np.int64)


def _run(rho, thetas, A_stack, B_stack, C_stack, u_idx, p_idx, trace=False):
    rho = np.asarray(rho, np.float32)
    B_stack = np.asarray(B_stack)
    u_idx = np.asarray(u_idx).astype(np.int64)
    p_idx = np.asarray(p_idx).astype(np.int64)
    n = rho.shape[0]
    assert n == N, n

    if "struct" not in _cache:
        pairs = _gate_pairs(B_stack)
        _cache["struct"] = (pairs, _grouping(pairs, n))
    pairs, perm = _cache["struct"]

    V = _build_V(thetas, pairs, u_idx, p_idx, n).astype(np.float32)
    V_g = V[np.ix_(perm, perm)]
    rho_g = np.ascontiguousarray(rho[np.ix_(perm, perm)])

    # block-diagonality check (structure is fixed by the module definition)
    blocks = [
        V_g[j * BK : (j + 1) * BK, j * BK : (j + 1) * BK] for j in range(NB)
    ]
    bd = np.zeros_like(V_g)
    for j in range(NB):
        bd[j * BK : (j + 1) * BK, j * BK : (j + 1) * BK] = blocks[j]
    assert np.array_equal(bd, V_g), "V lost block-diagonal structure"

    # vtb: [124, 496], column block j = B_j^T (shared by all cores)
    vtb = np.ascontiguousarray(np.concatenate([b.T for b in blocks], axis=1))

    in_maps = []
    for k in range(NCORES):
        B, h = divmod(k, 2)
        in_maps.append(
            {
                "rho": np.ascontiguousarray(rho_g[B * BK : (B + 1) * BK, :]),
                "vkt": np.ascontiguousarray(
                    blocks[B][h * R : (h + 1) * R, :].T
                ),
                "vtb": vtb,
            }
        )

    res = run_bass_kernel_spmd(_program(), in_maps, list(range(NCORES)), trace=trace)
    out_g = np.concatenate([res.results[k]["out"] for k in range(NCORES)], axis=0)
    out = np.empty((n, n), np.float32)
    out[np.ix_(perm, perm)] = out_g
    return out, res


def kernel(rho, thetas, A_stack, B_stack, C_stack, u_idx, p_idx):
    out, _ = _run(rho, thetas, A_stack, B_stack, C_stack, u_idx, p_idx)
    return out
